# revision 12
# baseline (speedup 1.0000x reference)
"""Paged GQA decode attention (B=64, HQ=32, HKV=8, D=128) on 8 TRN2 NeuronCores.

Strategy: data-parallel over requests with host-side load balancing.
 - Sort the 64 requests by context_lens descending; slot r of core c gets the
   rank-(r*8+c) request, so every core's slot-r request has a similar length.
 - Each slot is padded to the max-of-8 chunk count (chunks of 128 tokens), so
   all 8 cores execute the SAME static program (SPMD) on different data.
 - Host gathers each request's KV blocks (honoring block_tables) into per-core
   shards: K pre-transposed to [d, l] tiles (no on-chip transposes) in bf16;
   V in fp8e4m3-family (fp8e3 = e3m4) with a per-head ones/mask column
   appended, and invalid-token rows zeroed.  The mask column makes the PV
   matmul emit the softmax denominator for free, and the zeroed V rows mask
   padded/invalid tokens without any score bias.
 - Chunks stream in GRP-sized DMA groups that may span request slots.
 - Per chunk on device: 8 score matmuls (K_h^T stationary, q streaming) into
   a group-wide PSUM tile; one ScalarE exp per GROUP (constant -2 shift keeps
   E in bf16 range); 8 col-tiled PV matmuls per chunk accumulate
   acc[head-strip, D+1] per slot (last column = denominator).  Slot epilogue:
   one DVE PSUM->SBUF copy + one DMA.  Final division happens on host.
"""

import math
import os
import sys
from contextlib import ExitStack

import numpy as np
import ml_dtypes  # noqa: F401  (numpy bf16/fp8 dtypes)

for _p in ("/opt/trn_rl_repo", "/root/.axon_site/_ro/trn_rl_repo"):
    if os.path.isdir(_p) and _p not in sys.path:
        sys.path.insert(0, _p)
        break

import concourse.bass as bass  # noqa: F401
import concourse.tile as tile
from concourse import bacc, mybir
from concourse.bass_utils import run_bass_kernel_spmd

B, HQ, HKV, D, BS, MB = 64, 32, 8, 128, 16, 128
G = HQ // HKV              # 4 query heads per kv head
SCALE = 0.08838834764831845
NCORES = 8
SLOTS = B // NCORES        # 8 request slots per core
CHUNK = 128                # tokens per chunk (= SBUF partitions)
BPC = CHUNK // BS          # blocks per chunk = 8
ROW = HKV * D              # 1024 K elements per token row
DV = D + 1                 # V row per head incl. mask column
ROWV = HKV * DV            # 1032 V elements per token row
GRP = 4                    # chunks per DMA group (groups may span slots)
KV_BUFS = 8                # group tiles in flight
K_ENG = "gpsimd"           # DMA issue engine for K: gpsimd|sync|scalar
V_ENG = "gpsimd"           # DMA issue engine for V
K_DT = "bf16"              # K/q dtype: "f32" | "bf16" | "fp8"
V_DT = "fp8"               # V dtype: "f32" | "bf16" | "fp8"

last_results = None        # stashed BassKernelResults for test.py

_prog_cache = {}


def _mdt(name):
    return {"f32": mybir.dt.float32, "bf16": mybir.dt.bfloat16,
            "fp8": mybir.dt.float8e3}[name]


def _ndt(name):
    return mybir.dt.np(_mdt(name))


def _build_program(s_counts, dma_only=False):
    f32 = mybir.dt.float32
    kdt, vdt = _mdt(K_DT), _mdt(V_DT)
    edt = mybir.dt.bfloat16  # E (softmax numerator) dtype
    C_total = sum(s_counts)
    NG = C_total // GRP
    nc = bacc.Bacc()

    k_d = nc.declare_dram_parameter("k", [NG, D, GRP * ROW], kdt,
                                    isOutput=False)
    v_d = nc.declare_dram_parameter("v", [NG, CHUNK, GRP * ROWV], vdt,
                                    isOutput=False)
    qT_d = nc.declare_dram_parameter("qT", [D, SLOTS * HQ], kdt, isOutput=False)
    out_d = nc.declare_dram_parameter("out", [SLOTS, CHUNK, 2 * DV], f32,
                                      isOutput=True)

    EXP = mybir.ActivationFunctionType.Exp

    # chunk -> owning slot, first/last flags
    slot_of, first_of, last_of = [], [], []
    for r, S_r in enumerate(s_counts):
        for j in range(S_r):
            slot_of.append(r)
            first_of.append(j == 0)
            last_of.append(j == S_r - 1)

    with tile.TileContext(nc) as tc, ExitStack() as ctx:
        kpool = ctx.enter_context(tc.tile_pool(name="kp", bufs=KV_BUFS))
        vpool = ctx.enter_context(tc.tile_pool(name="vp", bufs=KV_BUFS))
        epool = ctx.enter_context(tc.tile_pool(name="e", bufs=3))
        opool = ctx.enter_context(tc.tile_pool(name="o", bufs=2))
        const = ctx.enter_context(tc.tile_pool(name="cst", bufs=1))
        spsum = ctx.enter_context(tc.tile_pool(name="sp", bufs=2, space="PSUM"))
        apsum = ctx.enter_context(tc.tile_pool(name="ac", bufs=2, space="PSUM"))

        q_all = const.tile([D, SLOTS * HQ], kdt)
        nc.sync.dma_start(q_all[:], qT_d[:])
        # dummy matmul absorbs the q_all DMA wait so the first real matmul
        # only waits on its k DMA (PE matmuls support only one sync wait).
        dmy = spsum.tile([1, 1], f32, tag="sco")
        nc.tensor.matmul(dmy[:], q_all[0:1, 0:1], q_all[0:1, 0:1],
                         start=True, stop=True)

        acc = None
        for g in range(NG):
            k_eng = K_ENG if g % 2 == 0 else "sync"
            v_eng = V_ENG if g % 2 == 0 else "scalar"
            kg = kpool.tile([D, GRP * ROW], kdt, tag="kg", name="kg")
            getattr(nc, k_eng).dma_start(kg[:], k_d[g])
            vg = vpool.tile([CHUNK, GRP * ROWV], vdt, tag="vg", name="vg")
            getattr(nc, v_eng).dma_start(vg[:], v_d[g])
            if dma_only:
                ot = opool.tile([CHUNK, 2 * DV], f32, tag="out")
                nc.vector.tensor_copy(ot[:, 0:DV], kg[:, 0:DV])
                nc.vector.tensor_copy(ot[:, DV:2 * DV], vg[:, 0:DV])
                if g % 8 == 7:
                    nc.sync.dma_start(out_d[slot_of[g * GRP]], ot[:])
                continue

            sco = spsum.tile([CHUNK, GRP * HQ], f32, tag="sco")
            for half in range(GRP):
                c = g * GRP + half
                r = slot_of[c]
                kt = kg[:, half * ROW:(half + 1) * ROW]
                for h in range(HKV):
                    nc.tensor.matmul(
                        sco[:, half * HQ + h * G:half * HQ + (h + 1) * G],
                        kt[:, h * D:(h + 1) * D],
                        q_all[:, r * HQ + h * G:r * HQ + (h + 1) * G],
                        start=True, stop=True,
                    )
            et = epool.tile([CHUNK, GRP * HQ], edt)
            nc.scalar.activation(et[:], sco[:], EXP, bias=0.0, scale=1.0)

            for half in range(GRP):
                c = g * GRP + half
                r = slot_of[c]
                if first_of[c]:
                    # separate banks per head-half: a start=True clears
                    # has_written for the whole bank on the written
                    # partitions, so the two halves must not share a bank.
                    acc_a = apsum.tile([CHUNK, DV], f32, tag="acca",
                                       name="acc_a")
                    acc_b = apsum.tile([CHUNK, DV], f32, tag="accb",
                                       name="acc_b")
                    acc = (acc_a, acc_b)
                st, sp = first_of[c], last_of[c]
                for h in range(HKV):
                    jj = h % G
                    nc.tensor.matmul(
                        acc[h // G][32 * jj:32 * jj + G, :],
                        et[:, half * HQ + h * G:half * HQ + (h + 1) * G],
                        vg[:, half * ROWV + h * DV:half * ROWV + (h + 1) * DV],
                        start=st, stop=sp,
                        tile_position=(0, 32 * jj),
                    )
                if last_of[c]:
                    ot = opool.tile([CHUNK, 2 * DV], f32, tag="out")
                    nc.vector.tensor_copy(ot[:, 0:DV], acc[0][:])
                    nc.vector.tensor_copy(ot[:, DV:2 * DV], acc[1][:])
                    nc.sync.dma_start(out_d[r], ot[:])
    nc.compile()
    return nc


def _get_program(s_counts):
    if s_counts not in _prog_cache:
        _prog_cache[s_counts] = _build_program(s_counts)
    return _prog_cache[s_counts]


def _make_schedule(context_lens):
    L = context_lens.astype(np.int64)
    order = np.argsort(-L, kind="stable")
    s_counts = []
    for r in range(SLOTS):
        grp = order[r * NCORES:(r + 1) * NCORES]
        s_counts.append(max(1, math.ceil(int(L[grp].max()) / CHUNK)))
    rem = (-sum(s_counts)) % GRP
    s_counts[-1] += rem  # pad stream so DMA groups tile it exactly
    return order, tuple(s_counts)


def _build_in_maps(q, k_cache, v_cache, block_tables, L, order, s_counts):
    np_k, np_v = _ndt(K_DT), _ndt(V_DT)
    C_total = sum(s_counts)
    nblocks_total = k_cache.shape[0]
    kf = k_cache.reshape(nblocks_total, BS, ROW)
    vf = v_cache.reshape(nblocks_total, BS, HKV, D)

    in_maps = []
    core_reqs = []
    for c in range(NCORES):
        karr = np.empty((C_total, D, ROW), np_k)
        varr = np.zeros((C_total, CHUNK, HKV, DV), np.float32)
        qT = np.empty((D, SLOTS * HQ), np_k)
        reqs = []
        gc = 0
        for r in range(SLOTS):
            b = int(order[r * NCORES + c])
            reqs.append(b)
            S_r = s_counts[r]
            blocks = np.clip(block_tables[b, :S_r * BPC].astype(np.int64),
                             0, nblocks_total - 1)
            kreq = kf[blocks].reshape(S_r, CHUNK, HKV, D)
            karr[gc:gc + S_r] = \
                kreq.transpose(0, 3, 2, 1).reshape(S_r, D, ROW)
            nv = min(int(L[b]), S_r * CHUNK)
            vreq = vf[blocks].reshape(S_r * CHUNK, HKV, D)
            vslab = varr[gc:gc + S_r].reshape(S_r * CHUNK, HKV, DV)
            vslab[:nv, :, :D] = vreq[:nv]
            vslab[:nv, :, D] = 1.0
            qT[:, r * HQ:(r + 1) * HQ] = (q[b] * SCALE).T
            gc += S_r
        # repack into GRP-chunk DMA groups: partition-major within a group
        kg = np.ascontiguousarray(
            karr.reshape(C_total // GRP, GRP, D, ROW).transpose(0, 2, 1, 3)
        ).reshape(C_total // GRP, D, GRP * ROW)
        vg = np.ascontiguousarray(
            varr.reshape(C_total // GRP, GRP, CHUNK, ROWV)
            .transpose(0, 2, 1, 3)
        ).reshape(C_total // GRP, CHUNK, GRP * ROWV).astype(np_v)
        in_maps.append({"k": kg, "v": vg, "qT": qT})
        core_reqs.append(reqs)
    return in_maps, core_reqs


def kernel(q, k_cache, v_cache, block_tables, context_lens):
    global last_results
    q = np.asarray(q, dtype=np.float32)
    k_cache = np.asarray(k_cache, dtype=np.float32)
    v_cache = np.asarray(v_cache, dtype=np.float32)
    block_tables = np.asarray(block_tables, dtype=np.int32)
    context_lens = np.asarray(context_lens, dtype=np.int32)

    L = context_lens.astype(np.int64)
    order, s_counts = _make_schedule(context_lens)
    nc = _get_program(s_counts)
    in_maps, core_reqs = _build_in_maps(
        q, k_cache, v_cache, block_tables, L, order, s_counts)

    res = run_bass_kernel_spmd(
        nc, in_maps, list(range(NCORES)),
        trace=bool(os.environ.get("KBASS_TRACE")),
    )
    last_results = res

    out = np.empty((B, HQ, D), np.float32)
    for c in range(NCORES):
        raw = res.results[c]["out"].reshape(SLOTS, CHUNK, 2 * DV)
        for r, b in enumerate(core_reqs[c]):
            for h in range(HKV):
                jj = h % G
                blk = 0 if h < G else DV
                num = raw[r, 32 * jj:32 * jj + G, blk:blk + D]
                den = raw[r, 32 * jj:32 * jj + G, blk + D]
                out[b, h * G:(h + 1) * G] = \
                    num / np.maximum(den, 1e-30)[:, None]
    return out


# revision 13
# speedup vs baseline: 1.0289x; 1.0289x over previous
"""Paged GQA decode attention (B=64, HQ=32, HKV=8, D=128) on 8 TRN2 NeuronCores.

Strategy: data-parallel over requests with host-side load balancing.
 - Sort the 64 requests by context_lens descending; slot r of core c gets the
   rank-(r*8+c) request, so every core's slot-r request has a similar length.
 - Each slot is padded to the max-of-8 chunk count (chunks of 128 tokens), so
   all 8 cores execute the SAME static program (SPMD) on different data.
 - Host gathers each request's KV blocks (honoring block_tables) into per-core
   shards: K pre-transposed to [d, l] tiles (no on-chip transposes) in bf16;
   V in fp8e3 (e3m4) with a per-head ones/mask column appended and
   invalid-token rows zeroed.  The mask column makes the PV matmul emit the
   softmax denominator for free, and the zeroed V rows mask padded/invalid
   tokens without any score bias.
 - Chunks stream in variable-size DMA groups (ramp 1,1,2 then GRP-sized;
   groups may span request slots).  Small leading groups let the PE start
   ~8us earlier; the exact-fit last group avoids stream padding.
 - Per chunk on device: 8 score matmuls (K_h^T stationary, q streaming) into
   a group-wide PSUM tile; one ScalarE exp per GROUP; 8 col-tiled PV matmuls
   per chunk accumulate acc[head-strip, D+1] per slot (last column =
   denominator).  Slot epilogue: DVE PSUM->SBUF copies + one DMA.  Final
   division happens on host.
"""

import math
import os
import sys
from contextlib import ExitStack

import numpy as np
import ml_dtypes  # noqa: F401  (numpy bf16/fp8 dtypes)

for _p in ("/opt/trn_rl_repo", "/root/.axon_site/_ro/trn_rl_repo"):
    if os.path.isdir(_p) and _p not in sys.path:
        sys.path.insert(0, _p)
        break

import concourse.bass as bass  # noqa: F401
import concourse.tile as tile
from concourse import bacc, mybir
from concourse.bass_utils import run_bass_kernel_spmd

B, HQ, HKV, D, BS, MB = 64, 32, 8, 128, 16, 128
G = HQ // HKV              # 4 query heads per kv head
SCALE = 0.08838834764831845
NCORES = 8
SLOTS = B // NCORES        # 8 request slots per core
CHUNK = 128                # tokens per chunk (= SBUF partitions)
BPC = CHUNK // BS          # blocks per chunk = 8
ROW = HKV * D              # 1024 K elements per token row
DV = D + 1                 # V row per head incl. mask column
ROWV = HKV * DV            # 1032 V elements per token row
GRP = 4                    # max chunks per DMA group
RAMP = (1, 1, 2)           # leading group sizes for early PE start
KV_BUFS = 8                # group tiles in flight (per size class)
K_ENG = "gpsimd"           # DMA issue engine for K: gpsimd|sync|scalar
V_ENG = "gpsimd"           # DMA issue engine for V
K_DT = "bf16"              # K/q dtype: "f32" | "bf16" | "fp8"
V_DT = "fp8"               # V dtype: "f32" | "bf16" | "fp8"

last_results = None        # stashed BassKernelResults for test.py

_prog_cache = {}


def _mdt(name):
    return {"f32": mybir.dt.float32, "bf16": mybir.dt.bfloat16,
            "fp8": mybir.dt.float8e3}[name]


def _ndt(name):
    return mybir.dt.np(_mdt(name))


def _group_sizes(C_total):
    sizes = []
    rem = C_total
    for s in RAMP:
        if rem - s < GRP:
            break
        sizes.append(s)
        rem -= s
    while rem > GRP:
        sizes.append(GRP)
        rem -= GRP
    if rem:
        sizes.append(rem)
    return sizes


def _build_program(s_counts):
    f32 = mybir.dt.float32
    kdt, vdt = _mdt(K_DT), _mdt(V_DT)
    edt = mybir.dt.bfloat16  # E (softmax numerator) dtype
    C_total = sum(s_counts)
    sizes = _group_sizes(C_total)
    nc = bacc.Bacc()

    k_ds, v_ds = [], []
    for g, sz in enumerate(sizes):
        k_ds.append(nc.declare_dram_parameter(
            f"k{g}", [D, sz * ROW], kdt, isOutput=False))
        v_ds.append(nc.declare_dram_parameter(
            f"v{g}", [CHUNK, sz * ROWV], vdt, isOutput=False))
    qT_d = nc.declare_dram_parameter("qT", [D, SLOTS * HQ], kdt, isOutput=False)
    out_d = nc.declare_dram_parameter("out", [SLOTS, CHUNK, 2 * DV], f32,
                                      isOutput=True)

    EXP = mybir.ActivationFunctionType.Exp

    # chunk -> owning slot, first/last flags
    slot_of, first_of, last_of = [], [], []
    for r, S_r in enumerate(s_counts):
        for j in range(S_r):
            slot_of.append(r)
            first_of.append(j == 0)
            last_of.append(j == S_r - 1)

    with tile.TileContext(nc) as tc, ExitStack() as ctx:
        kpool = ctx.enter_context(tc.tile_pool(name="kp", bufs=KV_BUFS))
        vpool = ctx.enter_context(tc.tile_pool(name="vp", bufs=KV_BUFS))
        epool = ctx.enter_context(tc.tile_pool(name="e", bufs=3))
        opool = ctx.enter_context(tc.tile_pool(name="o", bufs=2))
        const = ctx.enter_context(tc.tile_pool(name="cst", bufs=1))
        spsum = ctx.enter_context(tc.tile_pool(name="sp", bufs=2, space="PSUM"))
        apsum = ctx.enter_context(tc.tile_pool(name="ac", bufs=2, space="PSUM"))

        q_all = const.tile([D, SLOTS * HQ], kdt)
        nc.sync.dma_start(q_all[:], qT_d[:])
        # dummy matmul absorbs the q_all DMA wait so the first real matmul
        # only waits on its k DMA (PE matmuls support only one sync wait).
        dmy = spsum.tile([1, 1], f32, tag="sco")
        nc.tensor.matmul(dmy[:], q_all[0:1, 0:1], q_all[0:1, 0:1],
                         start=True, stop=True)

        acc = None
        base = 0
        for g, sz in enumerate(sizes):
            kg = kpool.tile([D, sz * ROW], kdt, tag=f"kg{sz}", name="kg")
            getattr(nc, K_ENG).dma_start(kg[:], k_ds[g][:])
            vg = vpool.tile([CHUNK, sz * ROWV], vdt, tag=f"vg{sz}", name="vg")
            getattr(nc, V_ENG).dma_start(vg[:], v_ds[g][:])

            sco = spsum.tile([CHUNK, GRP * HQ], f32, tag="sco")
            for half in range(sz):
                r = slot_of[base + half]
                kt = kg[:, half * ROW:(half + 1) * ROW]
                for h in range(HKV):
                    nc.tensor.matmul(
                        sco[:, half * HQ + h * G:half * HQ + (h + 1) * G],
                        kt[:, h * D:(h + 1) * D],
                        q_all[:, r * HQ + h * G:r * HQ + (h + 1) * G],
                        start=True, stop=True,
                    )
            et = epool.tile([CHUNK, GRP * HQ], edt)
            nc.scalar.activation(et[:, 0:sz * HQ], sco[:, 0:sz * HQ], EXP,
                                 bias=0.0, scale=1.0)

            for half in range(sz):
                c = base + half
                r = slot_of[c]
                if first_of[c]:
                    # separate banks per head-half: a start=True clears
                    # has_written for the whole bank on the written
                    # partitions, so the two halves must not share a bank.
                    acc_a = apsum.tile([CHUNK, DV], f32, tag="acca",
                                       name="acc_a")
                    acc_b = apsum.tile([CHUNK, DV], f32, tag="accb",
                                       name="acc_b")
                    acc = (acc_a, acc_b)
                st, sp = first_of[c], last_of[c]
                for h in range(HKV):
                    jj = h % G
                    nc.tensor.matmul(
                        acc[h // G][32 * jj:32 * jj + G, :],
                        et[:, half * HQ + h * G:half * HQ + (h + 1) * G],
                        vg[:, half * ROWV + h * DV:half * ROWV + (h + 1) * DV],
                        start=st, stop=sp,
                        tile_position=(0, 32 * jj),
                    )
                if last_of[c]:
                    ot = opool.tile([CHUNK, 2 * DV], f32, tag="out")
                    nc.vector.tensor_copy(ot[:, 0:DV], acc[0][:])
                    nc.vector.tensor_copy(ot[:, DV:2 * DV], acc[1][:])
                    nc.sync.dma_start(out_d[r], ot[:])
            base += sz
    nc.compile()
    return nc


def _get_program(s_counts):
    if s_counts not in _prog_cache:
        _prog_cache[s_counts] = _build_program(s_counts)
    return _prog_cache[s_counts]


def _make_schedule(context_lens):
    L = context_lens.astype(np.int64)
    order = np.argsort(-L, kind="stable")
    s_counts = []
    for r in range(SLOTS):
        grp = order[r * NCORES:(r + 1) * NCORES]
        s_counts.append(max(1, math.ceil(int(L[grp].max()) / CHUNK)))
    return order, tuple(s_counts)


def _build_in_maps(q, k_cache, v_cache, block_tables, L, order, s_counts):
    np_k, np_v = _ndt(K_DT), _ndt(V_DT)
    C_total = sum(s_counts)
    sizes = _group_sizes(C_total)
    nblocks_total = k_cache.shape[0]
    kf = k_cache.reshape(nblocks_total, BS, ROW)
    vf = v_cache.reshape(nblocks_total, BS, HKV, D)

    in_maps = []
    core_reqs = []
    for c in range(NCORES):
        karr = np.empty((C_total, D, ROW), np_k)
        varr = np.zeros((C_total, CHUNK, HKV, DV), np.float32)
        qT = np.empty((D, SLOTS * HQ), np_k)
        reqs = []
        gc = 0
        for r in range(SLOTS):
            b = int(order[r * NCORES + c])
            reqs.append(b)
            S_r = s_counts[r]
            blocks = np.clip(block_tables[b, :S_r * BPC].astype(np.int64),
                             0, nblocks_total - 1)
            kreq = kf[blocks].reshape(S_r, CHUNK, HKV, D)
            karr[gc:gc + S_r] = \
                kreq.transpose(0, 3, 2, 1).reshape(S_r, D, ROW)
            nv = min(int(L[b]), S_r * CHUNK)
            vreq = vf[blocks].reshape(S_r * CHUNK, HKV, D)
            vslab = varr[gc:gc + S_r].reshape(S_r * CHUNK, HKV, DV)
            vslab[:nv, :, :D] = vreq[:nv]
            vslab[:nv, :, D] = 1.0
            qT[:, r * HQ:(r + 1) * HQ] = (q[b] * SCALE).T
            gc += S_r
        varr8 = varr.reshape(C_total, CHUNK, ROWV).astype(np_v)
        in_map = {"qT": qT}
        base = 0
        for g, sz in enumerate(sizes):
            in_map[f"k{g}"] = np.ascontiguousarray(
                karr[base:base + sz].transpose(1, 0, 2)
            ).reshape(D, sz * ROW)
            in_map[f"v{g}"] = np.ascontiguousarray(
                varr8[base:base + sz].transpose(1, 0, 2)
            ).reshape(CHUNK, sz * ROWV)
            base += sz
        in_maps.append(in_map)
        core_reqs.append(reqs)
    return in_maps, core_reqs


def kernel(q, k_cache, v_cache, block_tables, context_lens):
    global last_results
    q = np.asarray(q, dtype=np.float32)
    k_cache = np.asarray(k_cache, dtype=np.float32)
    v_cache = np.asarray(v_cache, dtype=np.float32)
    block_tables = np.asarray(block_tables, dtype=np.int32)
    context_lens = np.asarray(context_lens, dtype=np.int32)

    L = context_lens.astype(np.int64)
    order, s_counts = _make_schedule(context_lens)
    nc = _get_program(s_counts)
    in_maps, core_reqs = _build_in_maps(
        q, k_cache, v_cache, block_tables, L, order, s_counts)

    res = run_bass_kernel_spmd(
        nc, in_maps, list(range(NCORES)),
        trace=bool(os.environ.get("KBASS_TRACE")),
    )
    last_results = res

    out = np.empty((B, HQ, D), np.float32)
    for c in range(NCORES):
        raw = res.results[c]["out"].reshape(SLOTS, CHUNK, 2 * DV)
        for r, b in enumerate(core_reqs[c]):
            for h in range(HKV):
                jj = h % G
                blk = 0 if h < G else DV
                num = raw[r, 32 * jj:32 * jj + G, blk:blk + D]
                den = raw[r, 32 * jj:32 * jj + G, blk + D]
                out[b, h * G:(h + 1) * G] = \
                    num / np.maximum(den, 1e-30)[:, None]
    return out


# revision 14
# speedup vs baseline: 1.2914x; 1.2551x over previous
"""Paged GQA decode attention (B=64, HQ=32, HKV=8, D=128) on 8 TRN2 NeuronCores.

Strategy: data-parallel over requests with host-side load balancing.
 - Sort the 64 requests by context_lens descending; slot r of core c gets the
   rank-(r*8+c) request, so every core's slot-r request has a similar length.
 - Each slot is padded to the max-of-8 chunk count (chunks of 128 tokens), so
   all 8 cores execute the SAME static program (SPMD) on different data.
 - Host gathers each request's KV blocks (honoring block_tables) into per-core
   shards: K pre-transposed to [d, l] tiles (no on-chip transposes) in bf16;
   V in fp8e3 (e3m4) with a per-head ones/mask column appended and
   invalid-token rows zeroed.  The mask column makes the PV matmul emit the
   softmax denominator for free, and the zeroed V rows mask padded/invalid
   tokens without any score bias.
 - Chunks stream in variable-size DMA groups (ramp 1,1,2 then GRP-sized;
   groups may span request slots).  Small leading groups let the PE start
   ~8us earlier; the exact-fit last group avoids stream padding.
 - Per chunk on device: 8 score matmuls (K_h^T stationary, q streaming) into
   a group-wide PSUM tile; one ScalarE exp per GROUP; 8 col-tiled PV matmuls
   per chunk accumulate acc[head-strip, D+1] per slot (last column =
   denominator).  Slot epilogue: DVE PSUM->SBUF copies + one DMA.  Final
   division happens on host.
"""

import math
import os
import sys
from contextlib import ExitStack

import numpy as np
import ml_dtypes  # noqa: F401  (numpy bf16/fp8 dtypes)

for _p in ("/opt/trn_rl_repo", "/root/.axon_site/_ro/trn_rl_repo"):
    if os.path.isdir(_p) and _p not in sys.path:
        sys.path.insert(0, _p)
        break

import concourse.bass as bass  # noqa: F401
import concourse.tile as tile
from concourse import bacc, mybir
from concourse.bass_utils import run_bass_kernel_spmd

B, HQ, HKV, D, BS, MB = 64, 32, 8, 128, 16, 128
G = HQ // HKV              # 4 query heads per kv head
SCALE = 0.08838834764831845
NCORES = 8
SLOTS = B // NCORES        # 8 request slots per core
CHUNK = 128                # tokens per chunk (= SBUF partitions)
BPC = CHUNK // BS          # blocks per chunk = 8
ROW = HKV * D              # 1024 K elements per token row
DV = D + 1                 # V row per head incl. mask column
ROWV = HKV * DV            # 1032 V elements per token row
GRP = 4                    # max chunks per DMA group
RAMP = (1, 1, 2)           # leading group sizes for early PE start
KV_BUFS = 8                # group tiles in flight (per size class)
K_ENG = "gpsimd"           # DMA issue engine for K: gpsimd|sync|scalar
V_ENG = "gpsimd"           # DMA issue engine for V
K_DT = "fp8"              # K/q dtype: "f32" | "bf16" | "fp8"
V_DT = "fp8"               # V dtype: "f32" | "bf16" | "fp8"

last_results = None        # stashed BassKernelResults for test.py

_prog_cache = {}


def _mdt(name):
    return {"f32": mybir.dt.float32, "bf16": mybir.dt.bfloat16,
            "fp8": mybir.dt.float8e3}[name]


def _ndt(name):
    return mybir.dt.np(_mdt(name))


def _group_sizes(C_total):
    sizes = []
    rem = C_total
    for s in RAMP:
        if rem - s < GRP:
            break
        sizes.append(s)
        rem -= s
    while rem > GRP:
        sizes.append(GRP)
        rem -= GRP
    if rem:
        sizes.append(rem)
    return sizes


def _build_program(s_counts):
    f32 = mybir.dt.float32
    kdt, vdt = _mdt(K_DT), _mdt(V_DT)
    edt = mybir.dt.bfloat16  # E (softmax numerator) dtype
    C_total = sum(s_counts)
    sizes = _group_sizes(C_total)
    nc = bacc.Bacc()

    k_ds, v_ds = [], []
    for g, sz in enumerate(sizes):
        k_ds.append(nc.declare_dram_parameter(
            f"k{g}", [D, sz * ROW], kdt, isOutput=False))
        v_ds.append(nc.declare_dram_parameter(
            f"v{g}", [CHUNK, sz * ROWV], vdt, isOutput=False))
    qT_d = nc.declare_dram_parameter("qT", [D, SLOTS * HQ], edt, isOutput=False)
    out_d = nc.declare_dram_parameter("out", [SLOTS, CHUNK, 2 * DV], f32,
                                      isOutput=True)

    EXP = mybir.ActivationFunctionType.Exp

    # chunk -> owning slot, first/last flags
    slot_of, first_of, last_of = [], [], []
    for r, S_r in enumerate(s_counts):
        for j in range(S_r):
            slot_of.append(r)
            first_of.append(j == 0)
            last_of.append(j == S_r - 1)

    with tile.TileContext(nc) as tc, ExitStack() as ctx:
        kpool = ctx.enter_context(tc.tile_pool(name="kp", bufs=KV_BUFS))
        vpool = ctx.enter_context(tc.tile_pool(name="vp", bufs=KV_BUFS))
        epool = ctx.enter_context(tc.tile_pool(name="e", bufs=3))
        opool = ctx.enter_context(tc.tile_pool(name="o", bufs=2))
        const = ctx.enter_context(tc.tile_pool(name="cst", bufs=1))
        spsum = ctx.enter_context(tc.tile_pool(name="sp", bufs=2, space="PSUM"))
        apsum = ctx.enter_context(tc.tile_pool(name="ac", bufs=2, space="PSUM"))

        q_all = const.tile([D, SLOTS * HQ], edt)
        nc.sync.dma_start(q_all[:], qT_d[:])
        # dummy matmul absorbs the q_all DMA wait so the first real matmul
        # only waits on its k DMA (PE matmuls support only one sync wait).
        dmy = spsum.tile([1, 1], f32, tag="sco")
        nc.tensor.matmul(dmy[:], q_all[0:1, 0:1], q_all[0:1, 0:1],
                         start=True, stop=True)

        acc = None
        base = 0
        for g, sz in enumerate(sizes):
            kg = kpool.tile([D, sz * ROW], kdt, tag=f"kg{sz}", name="kg")
            getattr(nc, K_ENG).dma_start(kg[:], k_ds[g][:])
            vg = vpool.tile([CHUNK, sz * ROWV], vdt, tag=f"vg{sz}", name="vg")
            getattr(nc, V_ENG).dma_start(vg[:], v_ds[g][:])

            sco = spsum.tile([CHUNK, GRP * HQ], f32, tag="sco")
            for half in range(sz):
                r = slot_of[base + half]
                kt = kg[:, half * ROW:(half + 1) * ROW]
                for h in range(HKV):
                    nc.tensor.matmul(
                        sco[:, half * HQ + h * G:half * HQ + (h + 1) * G],
                        kt[:, h * D:(h + 1) * D],
                        q_all[:, r * HQ + h * G:r * HQ + (h + 1) * G],
                        start=True, stop=True,
                    )
            et = epool.tile([CHUNK, GRP * HQ], edt)
            nc.scalar.activation(et[:, 0:sz * HQ], sco[:, 0:sz * HQ], EXP,
                                 bias=0.0, scale=1.0)

            for half in range(sz):
                c = base + half
                r = slot_of[c]
                if first_of[c]:
                    # separate banks per head-half: a start=True clears
                    # has_written for the whole bank on the written
                    # partitions, so the two halves must not share a bank.
                    acc_a = apsum.tile([CHUNK, DV], f32, tag="acca",
                                       name="acc_a")
                    acc_b = apsum.tile([CHUNK, DV], f32, tag="accb",
                                       name="acc_b")
                    acc = (acc_a, acc_b)
                st, sp = first_of[c], last_of[c]
                for h in range(HKV):
                    jj = h % G
                    nc.tensor.matmul(
                        acc[h // G][32 * jj:32 * jj + G, :],
                        et[:, half * HQ + h * G:half * HQ + (h + 1) * G],
                        vg[:, half * ROWV + h * DV:half * ROWV + (h + 1) * DV],
                        start=st, stop=sp,
                        tile_position=(0, 32 * jj),
                    )
                if last_of[c]:
                    ot = opool.tile([CHUNK, 2 * DV], f32, tag="out")
                    nc.vector.tensor_copy(ot[:, 0:DV], acc[0][:])
                    nc.vector.tensor_copy(ot[:, DV:2 * DV], acc[1][:])
                    nc.sync.dma_start(out_d[r], ot[:])
            base += sz
    nc.compile()
    return nc


def _get_program(s_counts):
    if s_counts not in _prog_cache:
        _prog_cache[s_counts] = _build_program(s_counts)
    return _prog_cache[s_counts]


def _make_schedule(context_lens):
    L = context_lens.astype(np.int64)
    order = np.argsort(-L, kind="stable")
    s_counts = []
    for r in range(SLOTS):
        grp = order[r * NCORES:(r + 1) * NCORES]
        s_counts.append(max(1, math.ceil(int(L[grp].max()) / CHUNK)))
    return order, tuple(s_counts)


def _build_in_maps(q, k_cache, v_cache, block_tables, L, order, s_counts):
    np_k, np_v = _ndt(K_DT), _ndt(V_DT)
    C_total = sum(s_counts)
    sizes = _group_sizes(C_total)
    nblocks_total = k_cache.shape[0]
    kf = k_cache.reshape(nblocks_total, BS, ROW)
    vf = v_cache.reshape(nblocks_total, BS, HKV, D)

    in_maps = []
    core_reqs = []
    for c in range(NCORES):
        karr = np.empty((C_total, D, ROW), np_k)
        varr = np.zeros((C_total, CHUNK, HKV, DV), np.float32)
        qT = np.empty((D, SLOTS * HQ), mybir.dt.np(mybir.dt.bfloat16))
        reqs = []
        gc = 0
        for r in range(SLOTS):
            b = int(order[r * NCORES + c])
            reqs.append(b)
            S_r = s_counts[r]
            blocks = np.clip(block_tables[b, :S_r * BPC].astype(np.int64),
                             0, nblocks_total - 1)
            kreq = kf[blocks].reshape(S_r, CHUNK, HKV, D)
            karr[gc:gc + S_r] = \
                kreq.transpose(0, 3, 2, 1).reshape(S_r, D, ROW)
            nv = min(int(L[b]), S_r * CHUNK)
            vreq = vf[blocks].reshape(S_r * CHUNK, HKV, D)
            vslab = varr[gc:gc + S_r].reshape(S_r * CHUNK, HKV, DV)
            vslab[:nv, :, :D] = vreq[:nv]
            vslab[:nv, :, D] = 1.0
            qT[:, r * HQ:(r + 1) * HQ] = (q[b] * SCALE).T
            gc += S_r
        varr8 = varr.reshape(C_total, CHUNK, ROWV).astype(np_v)
        in_map = {"qT": qT}
        base = 0
        for g, sz in enumerate(sizes):
            in_map[f"k{g}"] = np.ascontiguousarray(
                karr[base:base + sz].transpose(1, 0, 2)
            ).reshape(D, sz * ROW)
            in_map[f"v{g}"] = np.ascontiguousarray(
                varr8[base:base + sz].transpose(1, 0, 2)
            ).reshape(CHUNK, sz * ROWV)
            base += sz
        in_maps.append(in_map)
        core_reqs.append(reqs)
    return in_maps, core_reqs


def kernel(q, k_cache, v_cache, block_tables, context_lens):
    global last_results
    q = np.asarray(q, dtype=np.float32)
    k_cache = np.asarray(k_cache, dtype=np.float32)
    v_cache = np.asarray(v_cache, dtype=np.float32)
    block_tables = np.asarray(block_tables, dtype=np.int32)
    context_lens = np.asarray(context_lens, dtype=np.int32)

    L = context_lens.astype(np.int64)
    order, s_counts = _make_schedule(context_lens)
    nc = _get_program(s_counts)
    in_maps, core_reqs = _build_in_maps(
        q, k_cache, v_cache, block_tables, L, order, s_counts)

    res = run_bass_kernel_spmd(
        nc, in_maps, list(range(NCORES)),
        trace=bool(os.environ.get("KBASS_TRACE")),
    )
    last_results = res

    out = np.empty((B, HQ, D), np.float32)
    for c in range(NCORES):
        raw = res.results[c]["out"].reshape(SLOTS, CHUNK, 2 * DV)
        for r, b in enumerate(core_reqs[c]):
            for h in range(HKV):
                jj = h % G
                blk = 0 if h < G else DV
                num = raw[r, 32 * jj:32 * jj + G, blk:blk + D]
                den = raw[r, 32 * jj:32 * jj + G, blk + D]
                out[b, h * G:(h + 1) * G] = \
                    num / np.maximum(den, 1e-30)[:, None]
    return out


# revision 15
# speedup vs baseline: 1.3896x; 1.0761x over previous
"""Paged GQA decode attention (B=64, HQ=32, HKV=8, D=128) on 8 TRN2 NeuronCores.

Strategy: data-parallel over requests with host-side load balancing.
 - Sort the 64 requests by context_lens descending; slot r of core c gets the
   rank-(r*8+c) request, so every core's slot-r request has a similar length.
 - Each slot is padded to the max-of-8 chunk count (chunks of 128 tokens), so
   all 8 cores execute the SAME static program (SPMD) on different data.
 - Host gathers each request's KV blocks (honoring block_tables) into per-core
   shards: K pre-transposed to [d, l] tiles (no on-chip transposes) in bf16;
   V in fp8e3 (e3m4) with a per-head ones/mask column appended and
   invalid-token rows zeroed.  The mask column makes the PV matmul emit the
   softmax denominator for free, and the zeroed V rows mask padded/invalid
   tokens without any score bias.
 - Chunks stream in variable-size DMA groups (ramp 1,1,2 then GRP-sized;
   groups may span request slots).  Small leading groups let the PE start
   ~8us earlier; the exact-fit last group avoids stream padding.
 - Per chunk on device: 8 score matmuls (K_h^T stationary, q streaming) into
   a group-wide PSUM tile; one ScalarE exp per GROUP; 8 col-tiled PV matmuls
   per chunk accumulate acc[head-strip, D+1] per slot (last column =
   denominator).  Slot epilogue: DVE PSUM->SBUF copies + one DMA.  Final
   division happens on host.
"""

import math
import os
import sys
from contextlib import ExitStack

import numpy as np
import ml_dtypes  # noqa: F401  (numpy bf16/fp8 dtypes)

for _p in ("/opt/trn_rl_repo", "/root/.axon_site/_ro/trn_rl_repo"):
    if os.path.isdir(_p) and _p not in sys.path:
        sys.path.insert(0, _p)
        break

import concourse.bass as bass  # noqa: F401
import concourse.tile as tile
from concourse import bacc, mybir
from concourse.bass_utils import run_bass_kernel_spmd

B, HQ, HKV, D, BS, MB = 64, 32, 8, 128, 16, 128
G = HQ // HKV              # 4 query heads per kv head
SCALE = 0.08838834764831845
NCORES = 8
SLOTS = B // NCORES        # 8 request slots per core
CHUNK = 128                # tokens per chunk (= SBUF partitions)
BPC = CHUNK // BS          # blocks per chunk = 8
ROW = HKV * D              # 1024 K elements per token row
DV = D + 1                 # V row per head incl. mask column
ROWV = HKV * DV            # 1032 V elements per token row
GRP = 8                    # max chunks per DMA group
RAMP = (1, 1, 2, 4)           # leading group sizes for early PE start
KV_BUFS = 5                # group tiles in flight (per size class)
K_ENG = "gpsimd"           # DMA issue engine for K: gpsimd|sync|scalar
V_ENG = "gpsimd"           # DMA issue engine for V
K_DT = "fp8"              # K/q dtype: "f32" | "bf16" | "fp8"
V_DT = "fp8"               # V dtype: "f32" | "bf16" | "fp8"

last_results = None        # stashed BassKernelResults for test.py

_prog_cache = {}


def _mdt(name):
    return {"f32": mybir.dt.float32, "bf16": mybir.dt.bfloat16,
            "fp8": mybir.dt.float8e3}[name]


def _ndt(name):
    return mybir.dt.np(_mdt(name))


def _group_sizes(C_total):
    sizes = []
    rem = C_total
    for s in RAMP:
        if rem - s < GRP:
            break
        sizes.append(s)
        rem -= s
    while rem > GRP:
        sizes.append(GRP)
        rem -= GRP
    if rem:
        sizes.append(rem)
    return sizes


def _build_program(s_counts):
    f32 = mybir.dt.float32
    kdt, vdt = _mdt(K_DT), _mdt(V_DT)
    edt = mybir.dt.bfloat16  # E (softmax numerator) dtype
    C_total = sum(s_counts)
    sizes = _group_sizes(C_total)
    nc = bacc.Bacc()

    k_ds, v_ds = [], []
    for g, sz in enumerate(sizes):
        k_ds.append(nc.declare_dram_parameter(
            f"k{g}", [D, sz * ROW], kdt, isOutput=False))
        v_ds.append(nc.declare_dram_parameter(
            f"v{g}", [CHUNK, sz * ROWV], vdt, isOutput=False))
    qT_d = nc.declare_dram_parameter("qT", [D, SLOTS * HQ], edt, isOutput=False)
    out_d = nc.declare_dram_parameter("out", [SLOTS, CHUNK, 2 * DV], f32,
                                      isOutput=True)

    EXP = mybir.ActivationFunctionType.Exp

    # chunk -> owning slot, first/last flags
    slot_of, first_of, last_of = [], [], []
    for r, S_r in enumerate(s_counts):
        for j in range(S_r):
            slot_of.append(r)
            first_of.append(j == 0)
            last_of.append(j == S_r - 1)

    with tile.TileContext(nc) as tc, ExitStack() as ctx:
        kpool = ctx.enter_context(tc.tile_pool(name="kp", bufs=KV_BUFS))
        vpool = ctx.enter_context(tc.tile_pool(name="vp", bufs=KV_BUFS))
        epool = ctx.enter_context(tc.tile_pool(name="e", bufs=3))
        opool = ctx.enter_context(tc.tile_pool(name="o", bufs=2))
        const = ctx.enter_context(tc.tile_pool(name="cst", bufs=1))
        spsum = ctx.enter_context(tc.tile_pool(name="sp", bufs=2, space="PSUM"))
        apsum = ctx.enter_context(tc.tile_pool(name="ac", bufs=2, space="PSUM"))

        q_all = const.tile([D, SLOTS * HQ], edt)
        nc.sync.dma_start(q_all[:], qT_d[:])
        # dummy matmul absorbs the q_all DMA wait so the first real matmul
        # only waits on its k DMA (PE matmuls support only one sync wait).
        dmy = spsum.tile([1, 1], f32, tag="sco")
        nc.tensor.matmul(dmy[:], q_all[0:1, 0:1], q_all[0:1, 0:1],
                         start=True, stop=True)

        acc = None
        base = 0
        for g, sz in enumerate(sizes):
            kg = kpool.tile([D, sz * ROW], kdt, tag=f"kg{sz}", name="kg")
            getattr(nc, K_ENG).dma_start(kg[:], k_ds[g][:])
            vg = vpool.tile([CHUNK, sz * ROWV], vdt, tag=f"vg{sz}", name="vg")
            getattr(nc, V_ENG).dma_start(vg[:], v_ds[g][:])

            sco = spsum.tile([CHUNK, GRP * HQ], f32, tag="sco")
            for half in range(sz):
                r = slot_of[base + half]
                kt = kg[:, half * ROW:(half + 1) * ROW]
                for h in range(HKV):
                    nc.tensor.matmul(
                        sco[:, half * HQ + h * G:half * HQ + (h + 1) * G],
                        kt[:, h * D:(h + 1) * D],
                        q_all[:, r * HQ + h * G:r * HQ + (h + 1) * G],
                        start=True, stop=True,
                    )
            et = epool.tile([CHUNK, GRP * HQ], edt)
            nc.scalar.activation(et[:, 0:sz * HQ], sco[:, 0:sz * HQ], EXP,
                                 bias=0.0, scale=1.0)

            for half in range(sz):
                c = base + half
                r = slot_of[c]
                if first_of[c]:
                    # separate banks per head-half: a start=True clears
                    # has_written for the whole bank on the written
                    # partitions, so the two halves must not share a bank.
                    acc_a = apsum.tile([CHUNK, DV], f32, tag="acca",
                                       name="acc_a")
                    acc_b = apsum.tile([CHUNK, DV], f32, tag="accb",
                                       name="acc_b")
                    acc = (acc_a, acc_b)
                st, sp = first_of[c], last_of[c]
                for h in range(HKV):
                    jj = h % G
                    nc.tensor.matmul(
                        acc[h // G][32 * jj:32 * jj + G, :],
                        et[:, half * HQ + h * G:half * HQ + (h + 1) * G],
                        vg[:, half * ROWV + h * DV:half * ROWV + (h + 1) * DV],
                        start=st, stop=sp,
                        tile_position=(0, 32 * jj),
                    )
                if last_of[c]:
                    ot = opool.tile([CHUNK, 2 * DV], f32, tag="out")
                    nc.vector.tensor_copy(ot[:, 0:DV], acc[0][:])
                    nc.vector.tensor_copy(ot[:, DV:2 * DV], acc[1][:])
                    nc.sync.dma_start(out_d[r], ot[:])
            base += sz
    nc.compile()
    return nc


def _get_program(s_counts):
    if s_counts not in _prog_cache:
        _prog_cache[s_counts] = _build_program(s_counts)
    return _prog_cache[s_counts]


def _make_schedule(context_lens):
    L = context_lens.astype(np.int64)
    order = np.argsort(-L, kind="stable")
    s_counts = []
    for r in range(SLOTS):
        grp = order[r * NCORES:(r + 1) * NCORES]
        s_counts.append(max(1, math.ceil(int(L[grp].max()) / CHUNK)))
    return order, tuple(s_counts)


def _build_in_maps(q, k_cache, v_cache, block_tables, L, order, s_counts):
    np_k, np_v = _ndt(K_DT), _ndt(V_DT)
    C_total = sum(s_counts)
    sizes = _group_sizes(C_total)
    nblocks_total = k_cache.shape[0]
    kf = k_cache.reshape(nblocks_total, BS, ROW)
    vf = v_cache.reshape(nblocks_total, BS, HKV, D)

    in_maps = []
    core_reqs = []
    for c in range(NCORES):
        karr = np.empty((C_total, D, ROW), np_k)
        varr = np.zeros((C_total, CHUNK, HKV, DV), np.float32)
        qT = np.empty((D, SLOTS * HQ), mybir.dt.np(mybir.dt.bfloat16))
        reqs = []
        gc = 0
        for r in range(SLOTS):
            b = int(order[r * NCORES + c])
            reqs.append(b)
            S_r = s_counts[r]
            blocks = np.clip(block_tables[b, :S_r * BPC].astype(np.int64),
                             0, nblocks_total - 1)
            kreq = kf[blocks].reshape(S_r, CHUNK, HKV, D)
            karr[gc:gc + S_r] = \
                kreq.transpose(0, 3, 2, 1).reshape(S_r, D, ROW)
            nv = min(int(L[b]), S_r * CHUNK)
            vreq = vf[blocks].reshape(S_r * CHUNK, HKV, D)
            vslab = varr[gc:gc + S_r].reshape(S_r * CHUNK, HKV, DV)
            vslab[:nv, :, :D] = vreq[:nv]
            vslab[:nv, :, D] = 1.0
            qT[:, r * HQ:(r + 1) * HQ] = (q[b] * SCALE).T
            gc += S_r
        varr8 = varr.reshape(C_total, CHUNK, ROWV).astype(np_v)
        in_map = {"qT": qT}
        base = 0
        for g, sz in enumerate(sizes):
            in_map[f"k{g}"] = np.ascontiguousarray(
                karr[base:base + sz].transpose(1, 0, 2)
            ).reshape(D, sz * ROW)
            in_map[f"v{g}"] = np.ascontiguousarray(
                varr8[base:base + sz].transpose(1, 0, 2)
            ).reshape(CHUNK, sz * ROWV)
            base += sz
        in_maps.append(in_map)
        core_reqs.append(reqs)
    return in_maps, core_reqs


def kernel(q, k_cache, v_cache, block_tables, context_lens):
    global last_results
    q = np.asarray(q, dtype=np.float32)
    k_cache = np.asarray(k_cache, dtype=np.float32)
    v_cache = np.asarray(v_cache, dtype=np.float32)
    block_tables = np.asarray(block_tables, dtype=np.int32)
    context_lens = np.asarray(context_lens, dtype=np.int32)

    L = context_lens.astype(np.int64)
    order, s_counts = _make_schedule(context_lens)
    nc = _get_program(s_counts)
    in_maps, core_reqs = _build_in_maps(
        q, k_cache, v_cache, block_tables, L, order, s_counts)

    res = run_bass_kernel_spmd(
        nc, in_maps, list(range(NCORES)),
        trace=bool(os.environ.get("KBASS_TRACE")),
    )
    last_results = res

    out = np.empty((B, HQ, D), np.float32)
    for c in range(NCORES):
        raw = res.results[c]["out"].reshape(SLOTS, CHUNK, 2 * DV)
        for r, b in enumerate(core_reqs[c]):
            for h in range(HKV):
                jj = h % G
                blk = 0 if h < G else DV
                num = raw[r, 32 * jj:32 * jj + G, blk:blk + D]
                den = raw[r, 32 * jj:32 * jj + G, blk + D]
                out[b, h * G:(h + 1) * G] = \
                    num / np.maximum(den, 1e-30)[:, None]
    return out
